# revision 14
# baseline (speedup 1.0000x reference)
"""DepthMoE fused Trainium2 kernel (8-core SPMD, data-parallel over tokens).

Math (TOP_K=1 collapses the reference):
  gates = one_hot(argmax(x@w_gate));  log(sum_e exp(delta)*gates) = delta[e*]
  out = feats + scale * ((df_sel + x + (1-a0)*wt_b) @ wd_w.T + wd_b)
  df_sel@wd_w.T = (attn_sel[1:] @ A_e[1:]) @ W3_e,  W3_e = B_e@wt_w.T@wd_w.T
  (rank-16 contraction: never expand to the 100-token val matrix on-chip).

Per 128-token tile:
  pgt   = [B-blockdiag | 16*w_gate]^T @ x^T          -> gt [96,128], lg [6,128]
  S^T_e = A_e^T-block @ gt                           -> pst [100, 6*128]
  awt8  = exp(S^T/32)  (fp8, l on partitions)        -> [128, 6, 128]
  q     = awt8^T @ BDA (16A blockdiag | l0-ind | Z)  -> psq [128, 112]
  f6    = one_hot(argmax(lg^T)) / (16 Z);  q *= f6 (one broadcast mult)
  qs/awq: cast+transpose q back to lhsT form (ones row folds wd_b+u)
  bh    = [awq | x^T] @ [16*W3U | 16*wd_w^T] (fp8 DR) -> ps_o [128, 1024]
  out   = xbf16 + (scale/16)*ps_o                     -> bf16 store

W3/u are weight-only transforms computed on host (like the blockdiag packs).
5-stage software pipeline; PSUM = 8 banks exactly; PE never waits on
same-round producers in steady state.
"""

import numpy as np
import ml_dtypes

import concourse.bass as bass
import concourse.tile as tile
from concourse import bacc, mybir
from concourse.bass_utils import run_bass_kernel_spmd
from concourse.masks import make_identity

BF16 = mybir.dt.bfloat16
F32 = mybir.dt.float32
FP8 = mybir.dt.float8e4
NPBF16 = ml_dtypes.bfloat16
NPFP8 = ml_dtypes.float8_e4m3
DR = mybir.MatmulPerfMode.DoubleRow

NCORES = 8
TOK = 1024          # tokens per core
C = 1024
E, L, R = 6, 100, 16
NT = TOK // 128     # token tiles per core
CCH = C // 128      # contraction chunks of c
INV_SQRT_C = 1.0 / 32.0
GW = 96             # gt width: 96 B-blockdiag cols
QW = 112            # q psum width: 6*17 + 6 Z + 4 pad

TRACE = False
LAST_RESULTS = None

import os as _os


def _build_nc():
    nc = bacc.Bacc("TRN2", target_bir_lowering=False, debug=False,
                   num_devices=NCORES)

    d_xt8 = nc.dram_tensor("xt8", [128, NT * CCH * 128], FP8,
                           kind="ExternalInput")
    d_xbf = nc.dram_tensor("xbf", [TOK, C], BF16, kind="ExternalInput")
    d_bcpw8 = nc.dram_tensor("bcpw8", [128, CCH * GW], FP8,
                             kind="ExternalInput")
    d_wg8 = nc.dram_tensor("wg8", [128, CCH * 8], FP8, kind="ExternalInput")
    d_atblk = nc.dram_tensor("atblk", [96, E * L], BF16, kind="ExternalInput")
    d_bda8 = nc.dram_tensor("bda8", [128, E * QW], FP8, kind="ExternalInput")
    d_w3u8 = nc.dram_tensor("w3u8", [128, 2 * C], FP8, kind="ExternalInput")
    d_wdw8 = nc.dram_tensor("wdw8", [128, CCH * C], FP8, kind="ExternalInput")
    d_scale = nc.dram_tensor("scale", [1, 1], F32, kind="ExternalInput")
    d_out = nc.dram_tensor("out", [TOK, C], BF16, kind="ExternalOutput")

    with tile.TileContext(nc) as tc:
        with (
            tc.tile_pool(name="const", bufs=1) as const,
            tc.tile_pool(name="gtp", bufs=2) as gtp,
            tc.tile_pool(name="wk", bufs=8) as wk,
            tc.tile_pool(name="g6p", bufs=2) as g6p,
            tc.tile_pool(name="ffp", bufs=6) as ffp,
            tc.tile_pool(name="obp", bufs=2) as obp,
            tc.tile_pool(name="p_gt", bufs=1, space="PSUM") as p_gt,
            tc.tile_pool(name="p_st", bufs=1, space="PSUM") as p_st,
            tc.tile_pool(name="p_q", bufs=1, space="PSUM") as p_q,
            tc.tile_pool(name="p_lg", bufs=1, space="PSUM") as p_lg,
            tc.tile_pool(name="p_qt", bufs=1, space="PSUM") as p_qt,
            tc.tile_pool(name="p_o", bufs=1, space="PSUM") as p_o,
        ):
            # --- resident operands -----------------------------------------
            BCPW8 = const.tile([128, CCH, GW], FP8)
            nc.sync.dma_start(out=BCPW8, in_=d_bcpw8[:, :])
            WG8 = const.tile([128, CCH, 8], FP8)
            nc.sync.dma_start(out=WG8, in_=d_wg8[:, :])
            XT8 = const.tile([128, NT, CCH, 128], FP8)
            for t in range(NT):
                eng = nc.sync if t < 4 else nc.scalar
                eng.dma_start(out=XT8[:, t, :, :],
                              in_=d_xt8[:, t * CCH * 128:
                                        (t + 1) * CCH * 128])
            SCL = const.tile([128, 1], F32)
            nc.sync.dma_start(out=SCL, in_=d_scale[:, :].to_broadcast((128, 1)))

            ATBLK = const.tile([96, E, L], BF16)
            nc.gpsimd.dma_start(out=ATBLK, in_=d_atblk[:, :])
            BDA8 = const.tile([128, E, QW], FP8)
            nc.gpsimd.dma_start(out=BDA8, in_=d_bda8[:, :])
            W3U8 = const.tile([128, 2, C], FP8)
            nc.gpsimd.dma_start(out=W3U8, in_=d_w3u8[:, :])
            WDW8 = const.tile([128, CCH, C], FP8)
            nc.gpsimd.dma_start(out=WDW8, in_=d_wdw8[:, :])

            IDN = const.tile([128, 128], BF16)
            make_identity(nc, IDN)

            # manually-rotated triple buffers (pads preset once)
            AWT8 = const.tile([128, 3, E, 128], FP8)
            # engine partition offsets must be 32-aligned: zero 96:128 once;
            # exp rewrites rows 96:100 every tile.
            nc.vector.memset(AWT8[96:128, :, :, :], 0.0)
            QS = const.tile([128, 3, 128], BF16)
            nc.vector.memset(QS[:, :, 102:128], 0.0)
            nc.vector.memset(QS[:, :, 102:103], 0.0625)
            AWQ = const.tile([128, 3, 2, 128], FP8)
            nc.vector.memset(AWQ[:, :, 1, :], 0.0)

            gt_l = [None] * NT
            ff_l = [None] * NT
            g6_l = [None] * (NT // 4)

            for r in range(NT + 4):
                # ---- stage 0: gate/score GEMM for tile r ----
                if r < NT:
                    ff = ffp.tile([128, C], BF16)
                    nc.scalar.dma_start(out=ff,
                                        in_=d_xbf[r * 128:(r + 1) * 128, :])
                    ff_l[r] = ff

                    pgt = p_gt.tile([GW, 128], F32, tag="gt")
                    for k in range(CCH // 2):
                        nc.tensor.matmul(
                            pgt, lhsT=BCPW8[:, 2 * k:2 * k + 2, :],
                            rhs=XT8[:, r, 2 * k:2 * k + 2, :],
                            start=(k == 0), stop=(k == CCH // 2 - 1),
                            perf_mode=DR)
                    # gpsimd cannot read PSUM: copies go to vector/scalar
                    gt = gtp.tile([96, 128], BF16)
                    nc.scalar.copy(out=gt, in_=pgt[0:96, :])
                    gt_l[r] = gt

                # ---- batched gate logits + argmax for group (r-1)//4 ----
                if r % 4 == 1 and r - 1 < NT:
                    g = (r - 1) // 4
                    plg = p_lg.tile([128, 4, 8], F32, tag="lg")
                    for tt in range(4):
                        for k in range(CCH // 2):
                            nc.tensor.matmul(
                                plg[:, tt, :],
                                lhsT=XT8[:, 4 * g + tt, 2 * k:2 * k + 2, :],
                                rhs=WG8[:, 2 * k:2 * k + 2, :],
                                start=(k == 0), stop=(k == CCH // 2 - 1),
                                perf_mode=DR)
                    mxb = wk.tile([128, 4], F32)
                    nc.vector.reduce_max(mxb, plg[:, :, 0:6],
                                         axis=mybir.AxisListType.X)
                    g6b = g6p.tile([128, 4, 6], F32)
                    nc.vector.tensor_tensor(
                        g6b, plg[:, :, 0:6],
                        mxb.rearrange("p (a o) -> p a o", o=1)
                        .to_broadcast((128, 4, 6)),
                        mybir.AluOpType.is_equal)
                    g6_l[g] = g6b

                # ---- stage 1: scores^T + exp for tile r-1 ----
                t1 = r - 1
                if 0 <= t1 < NT:
                    pst = p_st.tile([L, E * 128], F32, tag="st")
                    for e in range(E):
                        nc.tensor.matmul(
                            pst[:, e * 128:(e + 1) * 128],
                            lhsT=ATBLK[:, e, :], rhs=gt_l[t1],
                            start=True, stop=True)
                    nc.scalar.activation(
                        out=AWT8[0:L, t1 % 3, :, :],
                        in_=pst.rearrange("p (e t) -> p e t", e=E),
                        func=mybir.ActivationFunctionType.Exp,
                        scale=INV_SQRT_C)

                # ---- stage 2: q GEMM + gating for tile r-2 ----
                t2 = r - 2
                if 0 <= t2 < NT:
                    psq = p_q.tile([128, QW], F32, tag="q")
                    for j in range(E // 2):
                        nc.tensor.matmul(
                            psq, lhsT=AWT8[:, t2 % 3, 2 * j:2 * j + 2, :],
                            rhs=BDA8[:, 2 * j:2 * j + 2, :],
                            start=(j == 0), stop=(j == E // 2 - 1),
                            perf_mode=DR)
                    rz = wk.tile([128, 6], F32)
                    nc.vector.reciprocal(rz, psq[:, 102:108])
                    f6 = wk.tile([128, 6], F32)
                    nc.gpsimd.tensor_mul(f6, g6_l[t2 // 4][:, t2 % 4, :], rz)
                    # fused gate-scale + cast + copy psum->QS in one vector op
                    qv = psq[:, 0:102].rearrange("p (e c) -> p e c", e=E)
                    qso = QS[:, t2 % 3, 0:102].rearrange(
                        "p (e c) -> p e c", e=E)
                    f6b = f6.rearrange("p (e o) -> p e o", o=1) \
                        .to_broadcast((128, E, 17))
                    nc.vector.tensor_mul(qso, qv, f6b)

                # ---- stage 3: transpose q to lhsT form for tile r-3 ----
                t3 = r - 3
                if 0 <= t3 < NT:
                    pqt = p_qt.tile([128, 128], BF16, tag="qt")
                    nc.tensor.transpose(out=pqt, in_=QS[:, t3 % 3, :],
                                        identity=IDN)
                    nc.vector.tensor_copy(out=AWQ[:, t3 % 3, 0, :],
                                          in_=pqt)

                # ---- stage 4: big GEMM + epilogue for tile r-4 ----
                t4 = r - 4
                if 0 <= t4 < NT:
                    ps_o = p_o.tile([128, C], F32, tag="o")
                    for h in range(2):
                        hs = slice(h * 512, (h + 1) * 512)
                        nc.tensor.matmul(
                            ps_o[:, hs], lhsT=AWQ[:, t4 % 3, :, :],
                            rhs=W3U8[:, :, hs],
                            start=True, stop=False, perf_mode=DR)
                    for k in range(CCH // 2):
                        for h in range(2):
                            hs = slice(h * 512, (h + 1) * 512)
                            nc.tensor.matmul(
                                ps_o[:, hs],
                                lhsT=XT8[:, t4, 2 * k:2 * k + 2, :],
                                rhs=WDW8[:, 2 * k:2 * k + 2, hs],
                                start=False, stop=(k == CCH // 2 - 1),
                                perf_mode=DR)
                    ob = obp.tile([128, C], BF16)
                    nc.vector.scalar_tensor_tensor(
                        out=ob[:, 0:672], in0=ps_o[:, 0:672],
                        scalar=SCL[:, 0:1], in1=ff_l[t4][:, 0:672],
                        op0=mybir.AluOpType.mult, op1=mybir.AluOpType.add)
                    nc.scalar.activation(
                        out=ob[:, 672:C], in_=ps_o[:, 672:C],
                        func=mybir.ActivationFunctionType.Copy, scale=SCL)
                    nc.gpsimd.tensor_add(ob[:, 672:C], ob[:, 672:C],
                                         ff_l[t4][:, 672:C])
                    nc.gpsimd.dma_start(
                        out=d_out[t4 * 128:(t4 + 1) * 128, :], in_=ob)

    nc.compile()
    return nc


_NC_CACHE = None


def _chmajor(a):
    # [C, X] -> [128, CCH*X]: row ch*128+p lands at [p, ch*X:(ch+1)*X]
    Xw = a.shape[1]
    return np.ascontiguousarray(
        a.reshape(CCH, 128, Xw).transpose(1, 0, 2).reshape(128, CCH * Xw))


def kernel(**inputs):
    global _NC_CACHE, LAST_RESULTS
    feats = np.asarray(inputs["feats"], np.float32)
    A = np.asarray(inputs["A"], np.float32)
    B = np.asarray(inputs["B"], np.float32)
    w_gate = np.asarray(inputs["w_gate"], np.float32)
    wt_w = np.asarray(inputs["wt_w"], np.float32)
    wt_b = np.asarray(inputs["wt_b"], np.float32)
    wd_w = np.asarray(inputs["wd_w"], np.float32)
    wd_b = np.asarray(inputs["wd_b"], np.float32)
    scale = np.asarray(inputs["scale"], np.float32)

    Bsz, N, Cin = feats.shape
    x = feats.reshape(-1, Cin)

    # ---- host-side weight-only transforms ----
    # W3[e] = B[e] @ wt_w.T @ wd_w.T  (rank-16 per expert), u = wd_w @ wt_b
    W2 = B @ wt_w.T                      # [E, R, C]
    W3 = W2 @ wd_w.T                     # [E, R, C]
    u = wd_w @ wt_b                      # [C]

    w3u = np.zeros((128, 2, C), np.float32)
    for e in range(E):
        w3u[17 * e:17 * e + R, 0, :] = 16.0 * W3[e]
        w3u[17 * e + R, 0, :] = -16.0 * u
    w3u[102, 0, :] = 256.0 * (wd_b + u)

    bda = np.zeros((128, E, QW), np.float32)
    for e in range(E):
        bda[1:L, e, 17 * e:17 * e + R] = 16.0 * A[e][1:L, :]
        bda[0, e, 17 * e + R] = 16.0
        bda[0:L, e, 102 + e] = 16.0

    atblk = np.zeros((96, E, L), np.float32)
    for e in range(E):
        atblk[16 * e:16 * e + R, e, :] = A[e].T

    bcpw = np.zeros((C, GW), np.float32)
    for e in range(E):
        bcpw[:, 16 * e:16 * e + R] = B[e].T
    wg = np.zeros((C, 8), np.float32)
    wg[:, 0:E] = 16.0 * w_gate

    shared = {
        "bcpw8": _chmajor(bcpw).astype(NPFP8),
        "wg8": _chmajor(wg).astype(NPFP8),
        "atblk": atblk.reshape(96, E * L).astype(NPBF16),
        "bda8": bda.reshape(128, E * QW).astype(NPFP8),
        "w3u8": w3u.reshape(128, 2 * C).astype(NPFP8),
        "wdw8": _chmajor(np.ascontiguousarray(16.0 * wd_w.T)).astype(NPFP8),
        "scale": (scale.reshape(1, 1) / 16.0).astype(np.float32),
    }
    in_maps = []
    for i in range(NCORES):
        xs = x[i * TOK:(i + 1) * TOK]
        xt = np.ascontiguousarray(xs.T)          # [C, TOK]
        xt8 = np.ascontiguousarray(
            xt.reshape(CCH, 128, NT, 128).transpose(1, 2, 0, 3)
        ).reshape(128, NT * CCH * 128)
        in_maps.append({
            "xt8": xt8.astype(NPFP8),
            "xbf": xs.astype(NPBF16),
            **shared,
        })

    if _NC_CACHE is None:
        _NC_CACHE = _build_nc()
    kw = {}
    if TRACE and _os.environ.get("KTMPDIR"):
        kw["tmpdir"] = _os.environ["KTMPDIR"]
    res = run_bass_kernel_spmd(_NC_CACHE, in_maps, list(range(NCORES)),
                               trace=TRACE, **kw)
    LAST_RESULTS = res
    out = np.concatenate([np.asarray(res.results[i]["out"], np.float32)
                          for i in range(NCORES)], axis=0)
    return out.reshape(Bsz, N, Cin)


# revision 15
# speedup vs baseline: 1.0383x; 1.0383x over previous
"""DepthMoE fused Trainium2 kernel (8-core SPMD, data-parallel over tokens).

Math (TOP_K=1 collapses the reference):
  gates = one_hot(argmax(x@w_gate));  log(sum_e exp(delta)*gates) = delta[e*]
  out = feats + scale * ((df_sel + x + (1-a0)*wt_b) @ wd_w.T + wd_b)
  df_sel@wd_w.T = (attn_sel[1:] @ A_e[1:]) @ W3_e,  W3_e = B_e@wt_w.T@wd_w.T
  (rank-16 contraction: never expand to the 100-token val matrix on-chip).

Per 128-token tile:
  pgt   = [B-blockdiag | 16*w_gate]^T @ x^T          -> gt [96,128], lg [6,128]
  S^T_e = A_e^T-block @ gt                           -> pst [100, 6*128]
  awt8  = exp(S^T/32)  (fp8, l on partitions)        -> [128, 6, 128]
  q     = awt8^T @ BDA (16A blockdiag | l0-ind | Z)  -> psq [128, 112]
  f6    = one_hot(argmax(lg^T)) / (16 Z);  q *= f6 (one broadcast mult)
  qs/awq: cast+transpose q back to lhsT form (ones row folds wd_b+u)
  bh    = [awq | x^T] @ [16*W3U | 16*wd_w^T] (fp8 DR) -> ps_o [128, 1024]
  out   = xbf16 + (scale/16)*ps_o                     -> bf16 store

W3/u are weight-only transforms computed on host (like the blockdiag packs).
5-stage software pipeline; PSUM = 8 banks exactly; PE never waits on
same-round producers in steady state.
"""

import numpy as np
import ml_dtypes

import concourse.bass as bass
import concourse.tile as tile
from concourse import bacc, mybir
from concourse.bass_utils import run_bass_kernel_spmd
from concourse.masks import make_identity

BF16 = mybir.dt.bfloat16
F32 = mybir.dt.float32
FP8 = mybir.dt.float8e4
NPBF16 = ml_dtypes.bfloat16
NPFP8 = ml_dtypes.float8_e4m3
DR = mybir.MatmulPerfMode.DoubleRow

NCORES = 8
TOK = 1024          # tokens per core
C = 1024
E, L, R = 6, 100, 16
NT = TOK // 128     # token tiles per core
CCH = C // 128      # contraction chunks of c
INV_SQRT_C = 1.0 / 32.0
GW = 96             # gt width: 96 B-blockdiag cols
QW = 112            # q psum width: 6*17 + 6 Z + 4 pad

TRACE = False
LAST_RESULTS = None

import os as _os


def _build_nc():
    nc = bacc.Bacc("TRN2", target_bir_lowering=False, debug=False,
                   num_devices=NCORES)

    d_xt8 = nc.dram_tensor("xt8", [128, NT * CCH * 128], FP8,
                           kind="ExternalInput")
    d_xbf = nc.dram_tensor("xbf", [TOK, C], BF16, kind="ExternalInput")
    d_bcpw8 = nc.dram_tensor("bcpw8", [128, CCH * GW], FP8,
                             kind="ExternalInput")
    d_wg8 = nc.dram_tensor("wg8", [128, CCH * 8], FP8, kind="ExternalInput")
    d_atblk = nc.dram_tensor("atblk", [96, E * L], BF16, kind="ExternalInput")
    d_bda8 = nc.dram_tensor("bda8", [128, E * QW], FP8, kind="ExternalInput")
    d_w3u8 = nc.dram_tensor("w3u8", [128, 2 * C], FP8, kind="ExternalInput")
    d_wdw8 = nc.dram_tensor("wdw8", [128, CCH * C], FP8, kind="ExternalInput")
    d_scale = nc.dram_tensor("scale", [1, 1], F32, kind="ExternalInput")
    d_out = nc.dram_tensor("out", [TOK, C], BF16, kind="ExternalOutput")

    with tile.TileContext(nc) as tc:
        with (
            tc.tile_pool(name="const", bufs=1) as const,
            tc.tile_pool(name="gtp", bufs=2) as gtp,
            tc.tile_pool(name="wk", bufs=8) as wk,
            tc.tile_pool(name="g6p", bufs=2) as g6p,
            tc.tile_pool(name="ffp", bufs=6) as ffp,
            tc.tile_pool(name="obp", bufs=2) as obp,
            tc.tile_pool(name="p_cmb", bufs=1, space="PSUM") as p_cmb,
            tc.tile_pool(name="p_st", bufs=1, space="PSUM") as p_st,
            tc.tile_pool(name="p_qt", bufs=1, space="PSUM") as p_qt,
            tc.tile_pool(name="p_o", bufs=1, space="PSUM") as p_o,
        ):
            # --- resident operands -----------------------------------------
            BCPW8 = const.tile([128, CCH, GW], FP8)
            nc.sync.dma_start(out=BCPW8, in_=d_bcpw8[:, :])
            WG8 = const.tile([128, CCH, 8], FP8)
            nc.sync.dma_start(out=WG8, in_=d_wg8[:, :])
            XT8 = const.tile([128, NT, CCH, 128], FP8)
            for t in range(NT):
                eng = nc.sync if t < 4 else nc.scalar
                eng.dma_start(out=XT8[:, t, :, :],
                              in_=d_xt8[:, t * CCH * 128:
                                        (t + 1) * CCH * 128])
            SCL = const.tile([128, 1], F32)
            nc.sync.dma_start(out=SCL, in_=d_scale[:, :].to_broadcast((128, 1)))

            ATBLK = const.tile([96, E, L], BF16)
            nc.gpsimd.dma_start(out=ATBLK, in_=d_atblk[:, :])
            BDA8 = const.tile([128, E, QW], FP8)
            nc.gpsimd.dma_start(out=BDA8, in_=d_bda8[:, :])
            W3U8 = const.tile([128, 2, C], FP8)
            nc.gpsimd.dma_start(out=W3U8, in_=d_w3u8[:, :])
            WDW8 = const.tile([128, CCH, C], FP8)
            nc.gpsimd.dma_start(out=WDW8, in_=d_wdw8[:, :])

            IDN = const.tile([128, 128], BF16)
            make_identity(nc, IDN)

            # manually-rotated triple buffers (pads preset once)
            AWT8 = const.tile([128, 3, E, 128], FP8)
            # engine partition offsets must be 32-aligned: zero 96:128 once;
            # exp rewrites rows 96:100 every tile.
            nc.vector.memset(AWT8[96:128, :, :, :], 0.0)
            QS = const.tile([128, 3, 128], BF16)
            nc.vector.memset(QS[:, :, 102:128], 0.0)
            nc.vector.memset(QS[:, :, 102:103], 0.0625)
            AWQ = const.tile([128, 3, 2, 128], FP8)
            nc.vector.memset(AWQ[:, :, 1, :], 0.0)

            gt_l = [None] * NT
            ff_l = [None] * NT
            g6_l = [None] * (NT // 4)

            # one PSUM bank shared by the three narrow outputs
            CMB = p_cmb.tile([128, 384], F32, tag="cmb")

            for r in range(NT + 4):
                # ---- stage 0: gate/score GEMM for tile r ----
                if r < NT:
                    ff = ffp.tile([128, C], BF16)
                    nc.scalar.dma_start(out=ff,
                                        in_=d_xbf[r * 128:(r + 1) * 128, :])
                    ff_l[r] = ff

                    pgt = CMB[0:GW, 0:128]
                    for k in range(CCH // 2):
                        nc.tensor.matmul(
                            pgt, lhsT=BCPW8[:, 2 * k:2 * k + 2, :],
                            rhs=XT8[:, r, 2 * k:2 * k + 2, :],
                            start=(k == 0), stop=(k == CCH // 2 - 1),
                            perf_mode=DR)
                    # gpsimd cannot read PSUM: copies go to vector/scalar
                    gt = gtp.tile([96, 128], BF16)
                    nc.scalar.copy(out=gt, in_=pgt[0:96, :])
                    gt_l[r] = gt

                # ---- batched gate logits + argmax for group (r-1)//4 ----
                if r % 4 == 1 and r - 1 < NT:
                    g = (r - 1) // 4
                    plg = CMB[:, 240:272].rearrange(
                        "p (a c) -> p a c", a=4)
                    for tt in range(4):
                        for k in range(CCH // 2):
                            nc.tensor.matmul(
                                plg[:, tt, :],
                                lhsT=XT8[:, 4 * g + tt, 2 * k:2 * k + 2, :],
                                rhs=WG8[:, 2 * k:2 * k + 2, :],
                                start=(k == 0), stop=(k == CCH // 2 - 1),
                                perf_mode=DR)
                    mxb = wk.tile([128, 4], F32)
                    nc.vector.reduce_max(mxb, plg[:, :, 0:6],
                                         axis=mybir.AxisListType.X)
                    g6b = g6p.tile([128, 4, 6], F32)
                    nc.vector.tensor_tensor(
                        g6b, plg[:, :, 0:6],
                        mxb.rearrange("p (a o) -> p a o", o=1)
                        .to_broadcast((128, 4, 6)),
                        mybir.AluOpType.is_equal)
                    g6_l[g] = g6b

                # ---- stage 1: scores^T + exp for tile r-1 ----
                t1 = r - 1
                if 0 <= t1 < NT:
                    pst = p_st.tile([L, E * 128], F32,
                                    tag=f"st{t1 % 2}")
                    for e in range(E):
                        nc.tensor.matmul(
                            pst[:, e * 128:(e + 1) * 128],
                            lhsT=ATBLK[:, e, :], rhs=gt_l[t1],
                            start=True, stop=True)
                    nc.scalar.activation(
                        out=AWT8[0:L, t1 % 3, :, :],
                        in_=pst.rearrange("p (e t) -> p e t", e=E),
                        func=mybir.ActivationFunctionType.Exp,
                        scale=INV_SQRT_C)

                # ---- stage 2: q GEMM + gating for tile r-2 ----
                t2 = r - 2
                if 0 <= t2 < NT:
                    psq = CMB[:, 128:240]
                    for j in range(E // 2):
                        nc.tensor.matmul(
                            psq, lhsT=AWT8[:, t2 % 3, 2 * j:2 * j + 2, :],
                            rhs=BDA8[:, 2 * j:2 * j + 2, :],
                            start=(j == 0), stop=(j == E // 2 - 1),
                            perf_mode=DR)
                    rz = wk.tile([128, 6], F32)
                    nc.vector.reciprocal(rz, psq[:, 102:108])
                    f6 = wk.tile([128, 6], F32)
                    nc.gpsimd.tensor_mul(f6, g6_l[t2 // 4][:, t2 % 4, :], rz)
                    # fused gate-scale + cast + copy psum->QS in one vector op
                    qv = psq[:, 0:102].rearrange("p (e c) -> p e c", e=E)
                    qso = QS[:, t2 % 3, 0:102].rearrange(
                        "p (e c) -> p e c", e=E)
                    f6b = f6.rearrange("p (e o) -> p e o", o=1) \
                        .to_broadcast((128, E, 17))
                    nc.vector.tensor_mul(qso, qv, f6b)

                # ---- stage 3: transpose q to lhsT form for tile r-3 ----
                t3 = r - 3
                if 0 <= t3 < NT:
                    pqt = p_qt.tile([128, 128], BF16, tag="qt")
                    nc.tensor.transpose(out=pqt, in_=QS[:, t3 % 3, :],
                                        identity=IDN)
                    nc.vector.tensor_copy(out=AWQ[:, t3 % 3, 0, :],
                                          in_=pqt)

                # ---- stage 4: big GEMM + epilogue for tile r-4 ----
                t4 = r - 4
                if 0 <= t4 < NT:
                    ps_o = p_o.tile([128, C], F32, tag="o")
                    for h in range(2):
                        hs = slice(h * 512, (h + 1) * 512)
                        nc.tensor.matmul(
                            ps_o[:, hs], lhsT=AWQ[:, t4 % 3, :, :],
                            rhs=W3U8[:, :, hs],
                            start=True, stop=False, perf_mode=DR)
                    for k in range(CCH // 2):
                        for h in range(2):
                            hs = slice(h * 512, (h + 1) * 512)
                            nc.tensor.matmul(
                                ps_o[:, hs],
                                lhsT=XT8[:, t4, 2 * k:2 * k + 2, :],
                                rhs=WDW8[:, 2 * k:2 * k + 2, hs],
                                start=False, stop=(k == CCH // 2 - 1),
                                perf_mode=DR)
                    ob = obp.tile([128, C], BF16)
                    nc.vector.scalar_tensor_tensor(
                        out=ob[:, 0:672], in0=ps_o[:, 0:672],
                        scalar=SCL[:, 0:1], in1=ff_l[t4][:, 0:672],
                        op0=mybir.AluOpType.mult, op1=mybir.AluOpType.add)
                    nc.scalar.activation(
                        out=ob[:, 672:C], in_=ps_o[:, 672:C],
                        func=mybir.ActivationFunctionType.Copy, scale=SCL)
                    nc.gpsimd.tensor_add(ob[:, 672:C], ob[:, 672:C],
                                         ff_l[t4][:, 672:C])
                    nc.gpsimd.dma_start(
                        out=d_out[t4 * 128:(t4 + 1) * 128, :], in_=ob)

    nc.compile()
    return nc


_NC_CACHE = None


def _chmajor(a):
    # [C, X] -> [128, CCH*X]: row ch*128+p lands at [p, ch*X:(ch+1)*X]
    Xw = a.shape[1]
    return np.ascontiguousarray(
        a.reshape(CCH, 128, Xw).transpose(1, 0, 2).reshape(128, CCH * Xw))


def kernel(**inputs):
    global _NC_CACHE, LAST_RESULTS
    feats = np.asarray(inputs["feats"], np.float32)
    A = np.asarray(inputs["A"], np.float32)
    B = np.asarray(inputs["B"], np.float32)
    w_gate = np.asarray(inputs["w_gate"], np.float32)
    wt_w = np.asarray(inputs["wt_w"], np.float32)
    wt_b = np.asarray(inputs["wt_b"], np.float32)
    wd_w = np.asarray(inputs["wd_w"], np.float32)
    wd_b = np.asarray(inputs["wd_b"], np.float32)
    scale = np.asarray(inputs["scale"], np.float32)

    Bsz, N, Cin = feats.shape
    x = feats.reshape(-1, Cin)

    # ---- host-side weight-only transforms ----
    # W3[e] = B[e] @ wt_w.T @ wd_w.T  (rank-16 per expert), u = wd_w @ wt_b
    W2 = B @ wt_w.T                      # [E, R, C]
    W3 = W2 @ wd_w.T                     # [E, R, C]
    u = wd_w @ wt_b                      # [C]

    w3u = np.zeros((128, 2, C), np.float32)
    for e in range(E):
        w3u[17 * e:17 * e + R, 0, :] = 16.0 * W3[e]
        w3u[17 * e + R, 0, :] = -16.0 * u
    w3u[102, 0, :] = 256.0 * (wd_b + u)

    bda = np.zeros((128, E, QW), np.float32)
    for e in range(E):
        bda[1:L, e, 17 * e:17 * e + R] = 16.0 * A[e][1:L, :]
        bda[0, e, 17 * e + R] = 16.0
        bda[0:L, e, 102 + e] = 16.0

    atblk = np.zeros((96, E, L), np.float32)
    for e in range(E):
        atblk[16 * e:16 * e + R, e, :] = A[e].T

    bcpw = np.zeros((C, GW), np.float32)
    for e in range(E):
        bcpw[:, 16 * e:16 * e + R] = B[e].T
    wg = np.zeros((C, 8), np.float32)
    wg[:, 0:E] = 16.0 * w_gate

    shared = {
        "bcpw8": _chmajor(bcpw).astype(NPFP8),
        "wg8": _chmajor(wg).astype(NPFP8),
        "atblk": atblk.reshape(96, E * L).astype(NPBF16),
        "bda8": bda.reshape(128, E * QW).astype(NPFP8),
        "w3u8": w3u.reshape(128, 2 * C).astype(NPFP8),
        "wdw8": _chmajor(np.ascontiguousarray(16.0 * wd_w.T)).astype(NPFP8),
        "scale": (scale.reshape(1, 1) / 16.0).astype(np.float32),
    }
    in_maps = []
    for i in range(NCORES):
        xs = x[i * TOK:(i + 1) * TOK]
        xt = np.ascontiguousarray(xs.T)          # [C, TOK]
        xt8 = np.ascontiguousarray(
            xt.reshape(CCH, 128, NT, 128).transpose(1, 2, 0, 3)
        ).reshape(128, NT * CCH * 128)
        in_maps.append({
            "xt8": xt8.astype(NPFP8),
            "xbf": xs.astype(NPBF16),
            **shared,
        })

    if _NC_CACHE is None:
        _NC_CACHE = _build_nc()
    kw = {}
    if TRACE and _os.environ.get("KTMPDIR"):
        kw["tmpdir"] = _os.environ["KTMPDIR"]
    res = run_bass_kernel_spmd(_NC_CACHE, in_maps, list(range(NCORES)),
                               trace=TRACE, **kw)
    LAST_RESULTS = res
    out = np.concatenate([np.asarray(res.results[i]["out"], np.float32)
                          for i in range(NCORES)], axis=0)
    return out.reshape(Bsz, N, Cin)
